# revision 3
# baseline (speedup 1.0000x reference)
# Trainium2 Bass kernel: dense MoE combine
#   out[b,l,d] = log( sum_e gates[b,e] * exp(xs[e,b,l,d]) )
# xs [8,128,96,512] f32, gates [128,8] f32 -> out [128,96,512] f32.
#
# Strategy (memory-bound, headroom via data parallelism):
#  - Shard batch across 8 cores: per core xs_c [8,16,96,512] (25.2 MB),
#    gates_c [16,8]. No communication needed.
#  - Per-core layout: partition p = b_local*8 + j where j indexes 8 blocks
#    of 12 consecutive l rows. Each partition holds data of exactly ONE
#    batch element, so the gate for (b,e) is a per-partition scalar.
#  - gates are folded into the exp as a bias: g*exp(x) = exp(x + log g),
#    using ACT's free affine (out = func(in*scale + bias)), bias being a
#    per-partition [128,1] AP. log(gates) is computed host-side (tiny).
#  - Accumulate over the 8 experts with fp32 tensor_tensor adds on DVE,
#    then one Ln on ACT, then DMA out. Exp+Ln share one ACT table set
#    (natural_log_exp_and_others -> single table load).
#  - Free dim (12*512 = 6144 cols) is chunked so DMA/ACT/DVE pipeline;
#    every DMA moves >=0.75 MB with >=6 KB contiguous per partition.

import os
from contextlib import ExitStack

import numpy as np

E, B, L, D = 8, 128, 96, 512
N_CORES = 8
B_LOC = B // N_CORES        # 16 batch elements per core
J = 8                       # l-blocks per batch element -> 16*8 = 128 partitions
L2 = L // J                 # 12 l-rows per block
CHUNK_L2 = int(os.environ.get("KERNEL_CHUNK_L2", "6"))  # l-rows per pipeline chunk
N_CHUNKS = L2 // CHUNK_L2
CH = CHUNK_L2 * D           # free-dim columns per chunk

_NC = None


def _build_nc():
    import concourse.bacc as bacc
    import concourse.mybir as mybir
    import concourse.tile as tile

    f32 = mybir.dt.float32
    AF = mybir.ActivationFunctionType

    nc = bacc.Bacc("TRN2", target_bir_lowering=False, debug=False,
                   num_devices=N_CORES)
    xs = nc.dram_tensor("xs", [E, B_LOC, L, D], f32, kind="ExternalInput").ap()
    lgb = nc.dram_tensor("lgb", [128, E], f32, kind="ExternalInput").ap()
    out = nc.dram_tensor("out", [B_LOC, L, D], f32, kind="ExternalOutput").ap()

    # [E, (b j), (l2 d)]: partition stride = 12*512 elems, unit col stride
    xs_v = xs.rearrange("e b (j l2) d -> e (b j) (l2 d)", j=J)
    out_v = out.rearrange("b (j l2) d -> (b j) (l2 d)", j=J)

    with tile.TileContext(nc) as tc, ExitStack() as ctx:
        const_pool = ctx.enter_context(tc.tile_pool(name="const", bufs=1))
        ld_pool = ctx.enter_context(tc.tile_pool(name="ld", bufs=5))
        acc_pool = ctx.enter_context(tc.tile_pool(name="acc", bufs=2))

        lgb_t = const_pool.tile([128, E], f32)
        nc.sync.dma_start(out=lgb_t[:], in_=lgb[:])

        for c in range(N_CHUNKS):
            cols = slice(c * CH, (c + 1) * CH)
            acc = acc_pool.tile([128, CH], f32)
            for e in range(E):
                t = ld_pool.tile([128, CH], f32)
                nc.sync.dma_start(out=t[:], in_=xs_v[e][:, cols])
                dst = acc if e == 0 else t
                nc.scalar.activation(dst[:], t[:], AF.Exp,
                                     bias=lgb_t[:, e:e + 1])
                if e > 0:
                    nc.vector.tensor_add(acc[:], acc[:], t[:])
            nc.scalar.activation(acc[:], acc[:], AF.Ln)
            nc.sync.dma_start(out=out_v[:, cols], in_=acc[:])
    nc.compile()
    return nc


def _get_nc():
    global _NC
    if _NC is None:
        _NC = _build_nc()
    return _NC


def _make_in_maps(xs, gates):
    xs = np.asarray(xs, dtype=np.float32)
    gates = np.asarray(gates, dtype=np.float32)
    lg = np.log(gates.astype(np.float64)).astype(np.float32)  # [B, E]
    in_maps = []
    for i in range(N_CORES):
        bs = slice(i * B_LOC, (i + 1) * B_LOC)
        xs_c = np.ascontiguousarray(xs[:, bs])              # [E, 16, 96, 512]
        lgb_c = np.ascontiguousarray(np.repeat(lg[bs], J, axis=0))  # [128, E]
        in_maps.append({"xs": xs_c, "lgb": lgb_c})
    return in_maps


def _run(xs, gates, trace=False, **trace_kwargs):
    from concourse.bass_utils import run_bass_kernel_spmd

    nc = _get_nc()
    in_maps = _make_in_maps(xs, gates)
    res = run_bass_kernel_spmd(nc, in_maps, list(range(N_CORES)),
                               trace=trace, **trace_kwargs)
    out = np.concatenate([res.results[i]["out"] for i in range(N_CORES)],
                         axis=0)  # [B, L, D]
    return out, res


def kernel(xs, gates):
    out, _ = _run(xs, gates, trace=False)
    return out
